# revision 11
# baseline (speedup 1.0000x reference)
"""MultiHeadGraphAttention kernel for 8 Trainium2 NeuronCores — v2.

Strategy (vs the v1 staged kernel): eliminate the per-tile DVE wall-build
bottleneck by making the PE consume a block-diagonal attention-weight matrix
(lhsT) that is assembled on-device with two batched DVE ops per chunk.

Sharding: device d owns src nodes [d*6256, (d+1)*6256) and ALL their edges
(~200K). Host packs nodes into "groups" of <=16 nodes; a group's A-edges
(dst < 25024) and B-edges (dst >= 25024) each fill 2 tiles (X-groups, for
high-degree nodes) or 1 tile (Y-groups) of 128 slots. Per group the PE runs
accumulating matmuls:
    psum[64 cols (4h x 16 nodes), 128 f] += W_tile.T @ Xg_tile
where W_tile[slot, h*16+g] = (gidx[slot]==g) * ee[h, slot] is built by DVE
(is_equal + mult with broadcast access patterns) and Xg is dma_gather'ed
(int16 idx; two base pointers into one table dodge the 32767 limit; gather
descriptor generation is spread over all 4 SWDGE queues / Q7 core pairs;
pad slots gather row 0 and are zeroed by ee=0).
Host precomputes ee = exp(-leaky_relu(score)) in bf16 and divides by the
rowsums / applies the diag weight w after gathering device outputs.
"""

import sys

sys.path.insert(0, "/opt/trn_rl_repo")

import ml_dtypes
import numpy as np

import concourse.bass as bass
import concourse.tile as tile
from concourse import bacc, mybir
from concourse.bass_utils import run_bass_kernel_spmd
from concourse.library_config import mlp

N_NODES = 50000
N_PAD = 50048
NDEV = 8
NPD = N_PAD // NDEV          # 6256 src nodes per device
HALF = 25024                 # A: dst in [0,25024), B: [25024,50048)
P = 128
F = 128
H = 4
G = 16                       # node columns per group
C = H * G                    # 64 psum cols per group
XSUM_MIN = 480               # switch X->Y when next 16 nodes sum below this

_last_results = None
_program_cache = {}
# Trailing-(-1) idx trimming is UNSAFE: the NRT decode side reserves DMA ring
# space from the static num_idxs ("keep in lockstep" in dma_gather.hpp); a Q7
# that pushes fewer descriptors desyncs the rings -> wild DMA -> device hang.
_TRIM_TRAILING = False


def _chunk_meta(chunks):
    """Per-chunk derived offsets. chunks: tuple of (is_x, ngroups)."""
    meta = []
    tile0 = idx0 = ee0 = gx0 = out0 = 0
    for is_x, ng in chunks:
        tiles = (4 if is_x else 2) * ng
        half_t = tiles // 2
        nps = -(-ng // 8)
        meta.append(dict(is_x=is_x, ng=ng, tiles=tiles, half_t=half_t,
                         nps=nps, tile0=tile0, idx0=idx0, ee0=ee0, gx0=gx0,
                         out0=out0))
        tile0 += tiles
        idx0 += tiles * 8          # idx cols (i16): tiles*128/16
        ee0 += tiles * H
        gx0 += tiles
        out0 += nps * 512
    return meta, tile0, idx0, ee0, gx0, out0


def _build_program(chunks):
    f32 = mybir.dt.float32
    bf16 = mybir.dt.bfloat16
    i16 = mybir.dt.int16
    meta, ntiles, idx_cols, ee_cols, gx_cols, out_cols = _chunk_meta(chunks)

    nc = bacc.Bacc("TRN2", target_bir_lowering=False, debug=False,
                   num_devices=NDEV, num_swdge_queues=4)

    xtab = nc.dram_tensor("xtab", [N_PAD, F], bf16, kind="ExternalInput").ap()
    idxt = nc.dram_tensor("idxt", [32, idx_cols], i16,
                          kind="ExternalInput").ap()
    eet = nc.dram_tensor("eet", [P, ee_cols], bf16, kind="ExternalInput").ap()
    gxt = nc.dram_tensor("gxt", [P, gx_cols], bf16, kind="ExternalInput").ap()
    iota = nc.dram_tensor("iota", [P, G], bf16, kind="ExternalInput").ap()
    outt = nc.dram_tensor("outt", [P, out_cols], bf16,
                          kind="ExternalOutput").ap()

    with tile.TileContext(nc) as tc:
        with (
            tc.tile_pool(name="const", bufs=1) as cpool,
            tc.tile_pool(name="blkin", bufs=6) as bpool,
            tc.tile_pool(name="gath", bufs=4) as gpool,
            tc.tile_pool(name="wbuf", bufs=3) as wpool,
            tc.tile_pool(name="fin", bufs=3) as fpool,
            tc.tile_pool(name="psum", bufs=2, space="PSUM") as pspool,
        ):
            nc.gpsimd.load_library(mlp)
            iota_sb = cpool.tile([P, G], bf16)
            nc.sync.dma_start(iota_sb[:], iota[:, :])

            for c, cm in enumerate(meta):
                is_x, ng, tiles = cm["is_x"], cm["ng"], cm["tiles"]
                half_t, nps = cm["half_t"], cm["nps"]
                hw = half_t * 8    # idx cols per side
                t0 = (half_t + 1) // 2
                # 4 half-gathers per chunk issued as A0,B0,A1,B1 on queues
                # 0-3: spreads Q7 desc-gen over all 4 core pairs every chunk,
                # and the first half of the chunk's groups has both operands
                # (A0+B0) after 2 calls - matmuls start at ~50% of the gather.
                # idx stream stays laid out [A0, A1, B0, B1].
                t1 = half_t - t0
                splits = [(0, 0, 0, t0, 0),
                          (1, 1, 0, t0, hw),
                          (2, 0, t0, t1, t0 * 8),
                          (3, 1, t0, t1, hw + t0 * 8)]
                splits = [s for s in splits if s[3] > 0]
                idx_sb = bpool.tile([P, 2 * hw], i16, tag="idx")
                for q, side, lo, nt, off in splits:
                    nc.sync.dma_start(
                        idx_sb[32 * q:32 * q + 32, off:off + nt * 8],
                        idxt[:, cm["idx0"] + off:cm["idx0"] + off + nt * 8])
                ee_sb = bpool.tile([P, tiles * H], bf16, tag="ee")
                nc.sync.dma_start(ee_sb[:],
                                  eet[:, cm["ee0"]:cm["ee0"] + tiles * H])
                gx_sb = bpool.tile([P, tiles], bf16, tag="gx")
                nc.sync.dma_start(gx_sb[:], gxt[:, cm["gx0"]:cm["gx0"] + tiles])

                xga = gpool.tile([P, half_t * F], bf16, tag="xga")
                xgb = gpool.tile([P, half_t * F], bf16, tag="xgb")
                for q, side, lo, nt, off in splits:
                    sb = xgb if side else xga
                    nc.gpsimd.dma_gather(
                        out_ap=sb[:, lo * F:(lo + nt) * F]
                            .rearrange("p (k f) -> p k f", k=nt),
                        in_ap=xtab[HALF:2 * HALF] if side else xtab[0:HALF],
                        idxs_ap=idx_sb[:, off:off + nt * 8],
                        num_idxs=nt * P,
                        num_idxs_reg=nt * P,
                        elem_size=F,
                        single_packet=False,
                        queue_num=q,
                    )

                # mask[p, t, g] = (iota[g] == gidx[p, t]) for all tiles
                mask = wpool.tile([P, tiles * G], bf16, tag="mask")
                nc.vector.tensor_tensor(
                    out=mask[:].rearrange("p (t g) -> p t g", t=tiles),
                    in0=iota_sb[:].unsqueeze(1).broadcast_to([P, tiles, G]),
                    in1=gx_sb[:].unsqueeze(2).broadcast_to([P, tiles, G]),
                    op=mybir.AluOpType.is_equal,
                )
                # w[p, t, h, g] = mask[p, t, g] * ee[p, t, h]
                wgt = wpool.tile([P, tiles * C], bf16, tag="wgt")
                nc.vector.tensor_tensor(
                    out=wgt[:].rearrange("p (t h g) -> p t h g",
                                         t=tiles, h=H),
                    in0=mask[:].rearrange("p (t g) -> p t g", t=tiles)
                        .unsqueeze(2).broadcast_to([P, tiles, H, G]),
                    in1=ee_sb[:].rearrange("p (t h) -> p t h", t=tiles)
                        .unsqueeze(3).broadcast_to([P, tiles, H, G]),
                    op=mybir.AluOpType.mult,
                )

                # X-chunk group j: A tiles 2j,2j+1; B tiles half_t+2j(+1)
                # Y-chunk group j: A tile j; B tile half_t+j
                # group j: psum tile q=j//8, wq=j%8,
                #   partitions (wq%2)*64 + (h*16+col), free (wq//2)*128 + f
                pss = [pspool.tile([P, 512], f32, tag=f"ps{q}",
                                   name=f"ps{q}")
                       for q in range(nps)]
                for g in range(ng):
                    q, wq = g // 8, g % 8
                    po, fo = (wq % 2) * C, (wq // 2) * F
                    out_ap = pss[q][po:po + C, fo:fo + F]
                    if is_x:
                        tids = [2 * g, 2 * g + 1,
                                half_t + 2 * g, half_t + 2 * g + 1]
                        srcs = [xga, xga, xgb, xgb]
                        offs = [2 * g, 2 * g + 1, 2 * g, 2 * g + 1]
                    else:
                        tids = [g, half_t + g]
                        srcs = [xga, xgb]
                        offs = [g, g]
                    for j, (t, sb, o) in enumerate(zip(tids, srcs, offs)):
                        nc.tensor.matmul(
                            out=out_ap,
                            lhsT=wgt[:, t * C:(t + 1) * C],
                            rhs=sb[:, o * F:(o + 1) * F],
                            start=(j == 0), stop=(j == len(tids) - 1),
                        )

                ow = nps * 512
                osb = fpool.tile([P, ow], bf16, tag="osb")
                for q in range(nps):
                    nc.scalar.copy(osb[:, q * 512:(q + 1) * 512], pss[q][:])
                nc.sync.dma_start(outt[:, cm["out0"]:cm["out0"] + ow], osb[:])
    nc.compile()
    return nc


def _pack_device(a_d, b_d, target_ngx=None):
    """Two-pointer greedy packing over desc-degree-sorted nodes.

    Returns (gid, col, modes): node n -> group gid[n], column col[n];
    modes[k] in {0 (X: 2A+2B tiles), 1 (Y: 1A+1B)}; X groups first.
    If target_ngx is given, the X phase is stretched to that many groups
    (keeps per-device group counts aligned with the shared schedule).
    """
    deg = a_d + b_d
    order = np.argsort(-deg, kind="stable")
    a_s, b_s = a_d[order], b_d[order]
    n = len(a_s)
    wind = np.concatenate([np.cumsum(a_s + b_s), np.full(G, deg.sum())])

    gid_s = np.empty(n, np.int64)
    col_s = np.empty(n, np.int64)
    modes = []
    h, t = 0, n - 1
    y_phase = False
    k = 0
    while h <= t:
        if not y_phase:
            nx_done = sum(1 for m in modes if m == 0)
            if target_ngx is not None:
                y_phase = nx_done >= target_ngx
            else:
                lo = wind[h - 1] if h > 0 else 0
                if wind[min(h + G - 1, n - 1)] - lo < XSUM_MIN:
                    y_phase = True
        cap = P if y_phase else 2 * P
        remA, remB, cols = cap, cap, 0
        while h <= t and cols < G and a_s[h] <= remA and b_s[h] <= remB:
            gid_s[h] = k
            col_s[h] = cols
            remA -= a_s[h]
            remB -= b_s[h]
            cols += 1
            h += 1
        while h <= t and cols < G and a_s[t] <= remA and b_s[t] <= remB:
            gid_s[t] = k
            col_s[t] = cols
            remA -= a_s[t]
            remB -= b_s[t]
            cols += 1
            t -= 1
        assert cols > 0
        modes.append(1 if y_phase else 0)
        k += 1
    gid = np.empty(n, np.int64)
    col = np.empty(n, np.int64)
    gid[order] = gid_s
    col[order] = col_s
    return gid, col, np.asarray(modes)


def kernel(x, w, a, edge_index):
    global _last_results
    x = np.asarray(x, dtype=np.float32)
    w = np.asarray(w, dtype=np.float32)
    a = np.asarray(a, dtype=np.float32)
    edge_index = np.asarray(edge_index)

    src = edge_index[0].astype(np.int64)
    dst = edge_index[1].astype(np.int64)

    # host: tiny projections + edge scores + ee (bf16, shared with rowsum)
    c_src = (w[:, 0, :] * a[:, :F, 0]).astype(np.float32)   # [H,F]
    c_dst = (w[:, 0, :] * a[:, F:, 0]).astype(np.float32)
    s_src = x @ c_src.T                                     # [N,H]
    s_dst = x @ c_dst.T
    score = s_src[src] + s_dst[dst]                         # [E,H]
    lk = np.where(score > 0, score, 0.2 * score)
    ee = np.exp(-lk, dtype=np.float32)                      # [E,H]
    ee_bf = ee.astype(ml_dtypes.bfloat16)
    ee64 = ee_bf.astype(np.float64)

    rs = np.zeros((H, N_PAD), np.float64)
    for h in range(H):
        rs[h] = np.bincount(src, weights=ee64[:, h], minlength=N_PAD)
    rs[rs == 0] = 1.0

    x_pad = np.zeros((N_PAD, F), np.float32)
    x_pad[:N_NODES] = x
    x_bf = np.ascontiguousarray(x_pad.astype(ml_dtypes.bfloat16))
    iota_np = np.broadcast_to(np.arange(G, dtype=np.float32), (P, G)
                              ).astype(ml_dtypes.bfloat16)

    isB = dst >= HALF
    degA = np.bincount(src, weights=~isB, minlength=N_PAD).astype(np.int64)
    degB = np.bincount(src, weights=isB, minlength=N_PAD).astype(np.int64)
    # a node whose per-side degree exceeds an X-group's side capacity could
    # never be placed; fail loudly instead of corrupting the packing
    assert degA.max() <= 2 * P and degB.max() <= 2 * P, \
        (degA.max(), degB.max())

    # ---- per-device packing ----
    # pass 1: natural X/Y split per device; pass 2: align every device to the
    # max X-group count so the shared schedule pads as little as possible
    ngx_nat = []
    for d in range(NDEV):
        lo = d * NPD
        _, _, modes = _pack_device(degA[lo:lo + NPD], degB[lo:lo + NPD])
        ngx_nat.append(int((modes == 0).sum()))
    ngx_tgt = max(ngx_nat)
    dev_pack = []
    ngx_max = ngy_max = 0
    for d in range(NDEV):
        lo = d * NPD
        gid, col, modes = _pack_device(degA[lo:lo + NPD], degB[lo:lo + NPD],
                                       target_ngx=ngx_tgt)
        ngx = int((modes == 0).sum())
        ngy = int((modes == 1).sum())
        ngx_max = max(ngx_max, ngx)
        ngy_max = max(ngy_max, ngy)
        dev_pack.append((gid, col, modes, ngx))

    # chunk schedule: full X-chunks of 16 groups (+ partial), then Y of 32
    chunks = []
    r = ngx_max
    while r > 0:
        chunks.append((True, min(16, r)))
        r -= min(16, r)
    r = ngy_max
    while r > 0:
        chunks.append((False, min(32, r)))
        r -= min(32, r)
    chunks = tuple(chunks)
    meta, ntiles, idx_cols, ee_cols, gx_cols, out_cols = _chunk_meta(chunks)
    # per-group (schedule-level) lookup tables
    sch_rows = []   # (is_x, chunk_idx, j_in_chunk)
    for ci, (is_x, ng) in enumerate(chunks):
        for j in range(ng):
            sch_rows.append((is_x, ci, j))
    sch_isx = np.array([r[0] for r in sch_rows])
    sch_ci = np.array([r[1] for r in sch_rows])
    sch_j = np.array([r[2] for r in sch_rows])
    m_tile0 = np.array([m["tile0"] for m in meta])
    m_half = np.array([m["half_t"] for m in meta])
    m_out0 = np.array([m["out0"] for m in meta])
    # X-group k (global order) must map to k-th X-slot of the schedule;
    # schedule lists X groups first, so global group id == schedule row.
    gA_base = np.where(sch_isx,
                       m_tile0[sch_ci] + 2 * sch_j,
                       m_tile0[sch_ci] + sch_j)
    gB_base = gA_base + m_half[sch_ci]
    g_out = m_out0[sch_ci] + (sch_j // 8) * 512 + ((sch_j % 8) // 2) * F
    g_prow = ((sch_j % 8) % 2) * C

    in_maps = []
    for d in range(NDEV):
        lo = d * NPD
        gid, col, modes, ngx = dev_pack[d]
        # device group id -> schedule row: X groups k -> k; Y groups k -> (k -
        # ngx) + ngx_max
        m = (src >= lo) & (src < lo + NPD)
        es = src[m] - lo
        ed = dst[m]
        eb = isB[m]
        eee = ee_bf[m]                       # [Ed, H] bf16
        sgid = np.where(modes == 0, np.arange(len(modes)),
                        np.arange(len(modes)) - ngx + ngx_max)
        egid = sgid[gid[es]]
        ecol = col[es]

        # rank of each edge within its (group, side) bucket
        okey = np.lexsort((eb, egid))
        ed_o, eb_o = ed[okey], eb[okey]
        egid_o, ecol_o = egid[okey], ecol[okey]
        eee_o = eee[okey]
        bucket = egid_o * 2 + eb_o
        bchange = np.flatnonzero(np.diff(bucket)) + 1
        starts = np.concatenate([[0], bchange])
        bid = np.zeros(len(bucket), np.int64)
        bid[bchange] = 1
        bid = np.cumsum(bid)
        rank = np.arange(len(bucket)) - starts[bid]

        base = np.where(eb_o, gB_base[egid_o], gA_base[egid_o])
        tt = base + (rank >> 7)
        p = rank & 127

        idx_flat = np.zeros((ntiles, P), np.int16)  # [tile, slot]
        idx_flat[tt, p] = (ed_o - np.where(eb_o, HALF, 0)).astype(np.int16)

        # -1 for the strictly-trailing pad of each gather call: the Q7 trims
        # trailing negatives, skipping those descriptors entirely.
        # wrap per gather call ([A0, A1, B0, B1] per chunk, matching the
        # device's 4-way split): flat (tile, slot) -> [i%16, i//16] x2 bands
        idx_cols_dev = np.empty((32, idx_cols), np.int16)
        colpos = 0
        for cm in meta:
            t0s = (cm["half_t"] + 1) // 2
            for side in range(2):
                base = cm["tile0"] + side * cm["half_t"]
                for lo, nt in ((0, t0s), (t0s, cm["half_t"] - t0s)):
                    if not nt:
                        continue
                    fl = idx_flat[base + lo:base + lo + nt].reshape(-1)
                    wq = fl.reshape(-1, 16).T    # [16, nt*8]
                    idx_cols_dev[:, colpos:colpos + nt * 8] = \
                        np.tile(wq, (2, 1))
                    colpos += nt * 8
        assert colpos == idx_cols

        ee_arr = np.zeros((P, ee_cols), ml_dtypes.bfloat16)
        ee_arr[p[:, None], (tt * H)[:, None] + np.arange(H)[None, :]] = eee_o
        gx_arr = np.zeros((P, gx_cols), np.float32)
        gx_arr[p, tt] = ecol_o.astype(np.float32)

        in_maps.append({
            "xtab": x_bf,
            "idxt": idx_cols_dev,
            "eet": np.ascontiguousarray(ee_arr),
            "gxt": gx_arr.astype(ml_dtypes.bfloat16),
            "iota": iota_np,
        })

    if chunks not in _program_cache:
        _program_cache[chunks] = _build_program(chunks)
    nc = _program_cache[chunks]

    res = run_bass_kernel_spmd(nc, in_maps, core_ids=list(range(NDEV)))
    _last_results = res

    # ---- decode ----
    out = np.empty((H, N_PAD, F), np.float32)
    hh = np.arange(H)
    for d in range(NDEV):
        gid, col, modes, ngx = dev_pack[d]
        r = res.results[d]["outt"].astype(np.float32)   # [P, out_cols]
        sgid = np.where(modes == 0, np.arange(len(modes)),
                        np.arange(len(modes)) - ngx + ngx_max)
        node_s = sgid[gid]                              # schedule row per node
        pn = g_prow[node_s] + col                       # h=0 partition row
        fn = g_out[node_s]                              # col base
        idx_p = pn[None, :, None] + (hh * G)[:, None, None]
        idx_f = fn[None, :, None] + np.arange(F)[None, None, :]
        out[:, d * NPD:(d + 1) * NPD, :] = \
            r[np.broadcast_to(idx_p, (H, NPD, F)),
              np.broadcast_to(idx_f, (H, NPD, F))]
    out *= w[:, 0, :][:, None, :]
    out /= rs[:, :, None].astype(np.float32)
    return np.ascontiguousarray(out[:, :N_NODES, :]).astype(np.float32)


# revision 13
# speedup vs baseline: 1.0135x; 1.0135x over previous
"""MultiHeadGraphAttention kernel for 8 Trainium2 NeuronCores — v2.

Strategy (vs the v1 staged kernel): eliminate the per-tile DVE wall-build
bottleneck by making the PE consume a block-diagonal attention-weight matrix
(lhsT) that is assembled on-device with two batched DVE ops per chunk.

Sharding: device d owns src nodes [d*6256, (d+1)*6256) and ALL their edges
(~200K). Host packs nodes into "groups" of <=16 nodes; a group's A-edges
(dst < 25024) and B-edges (dst >= 25024) each fill 2 tiles (X-groups, for
high-degree nodes) or 1 tile (Y-groups) of 128 slots. Per group the PE runs
accumulating matmuls:
    psum[64 cols (4h x 16 nodes), 128 f] += W_tile.T @ Xg_tile
where W_tile[slot, h*16+g] = (gidx[slot]==g) * ee[h, slot] is built by DVE
(is_equal + mult with broadcast access patterns) and Xg is dma_gather'ed
(int16 idx; two base pointers into one table dodge the 32767 limit; gather
descriptor generation is spread over all 4 SWDGE queues / Q7 core pairs;
pad slots gather row 0 and are zeroed by ee=0).
Host precomputes ee = exp(-leaky_relu(score)) in bf16 and divides by the
rowsums / applies the diag weight w after gathering device outputs.
"""

import sys

sys.path.insert(0, "/opt/trn_rl_repo")

import ml_dtypes
import numpy as np

import concourse.bass as bass
import concourse.tile as tile
from concourse import bacc, mybir
from concourse.bass_utils import run_bass_kernel_spmd
from concourse.library_config import mlp

N_NODES = 50000
N_PAD = 50048
NDEV = 8
NPD = N_PAD // NDEV          # 6256 src nodes per device
HALF = 25024                 # A: dst in [0,25024), B: [25024,50048)
P = 128
F = 128
H = 4
G = 16                       # node columns per group
C = H * G                    # 64 psum cols per group
XSUM_MIN = 480               # switch X->Y when next 16 nodes sum below this

_last_results = None
_program_cache = {}
# Trailing-(-1) idx trimming is UNSAFE: the NRT decode side reserves DMA ring
# space from the static num_idxs ("keep in lockstep" in dma_gather.hpp); a Q7
# that pushes fewer descriptors desyncs the rings -> wild DMA -> device hang.
_TRIM_TRAILING = False


def _chunk_meta(chunks):
    """Per-chunk derived offsets. chunks: tuple of (is_x, ngroups)."""
    meta = []
    tile0 = idx0 = ee0 = gx0 = out0 = 0
    for is_x, ng in chunks:
        tiles = (4 if is_x else 2) * ng
        half_t = tiles // 2
        nps = -(-ng // 8)
        meta.append(dict(is_x=is_x, ng=ng, tiles=tiles, half_t=half_t,
                         nps=nps, tile0=tile0, idx0=idx0, ee0=ee0, gx0=gx0,
                         out0=out0))
        tile0 += tiles
        idx0 += tiles * 8          # idx cols (i16): tiles*128/16
        ee0 += tiles * H
        gx0 += tiles
        out0 += nps * 512
    return meta, tile0, idx0, ee0, gx0, out0


def _build_program(chunks):
    f32 = mybir.dt.float32
    bf16 = mybir.dt.bfloat16
    i16 = mybir.dt.int16
    meta, ntiles, idx_cols, ee_cols, gx_cols, out_cols = _chunk_meta(chunks)

    nc = bacc.Bacc("TRN2", target_bir_lowering=False, debug=False,
                   num_devices=NDEV, num_swdge_queues=4)

    xtab = nc.dram_tensor("xtab", [N_PAD, F], bf16, kind="ExternalInput").ap()
    idxt = nc.dram_tensor("idxt", [32, idx_cols], i16,
                          kind="ExternalInput").ap()
    eet = nc.dram_tensor("eet", [P, ee_cols], bf16, kind="ExternalInput").ap()
    gxt = nc.dram_tensor("gxt", [P, gx_cols], bf16, kind="ExternalInput").ap()
    iota = nc.dram_tensor("iota", [P, G], bf16, kind="ExternalInput").ap()
    outt = nc.dram_tensor("outt", [P, out_cols], bf16,
                          kind="ExternalOutput").ap()

    with tile.TileContext(nc) as tc:
        with (
            tc.tile_pool(name="const", bufs=1) as cpool,
            tc.tile_pool(name="blkin", bufs=6) as bpool,
            tc.tile_pool(name="gath", bufs=4) as gpool,
            tc.tile_pool(name="wbuf", bufs=3) as wpool,
            tc.tile_pool(name="fin", bufs=3) as fpool,
            tc.tile_pool(name="psum", bufs=2, space="PSUM") as pspool,
        ):
            nc.gpsimd.load_library(mlp)
            iota_sb = cpool.tile([P, G], bf16)
            nc.sync.dma_start(iota_sb[:], iota[:, :])

            for c, cm in enumerate(meta):
                is_x, ng, tiles = cm["is_x"], cm["ng"], cm["tiles"]
                half_t, nps = cm["half_t"], cm["nps"]
                hw = half_t * 8    # idx cols per side
                t0 = (half_t + 1) // 2
                # 4 half-gathers per chunk issued as A0,B0,A1,B1 on queues
                # 0-3: spreads Q7 desc-gen over all 4 core pairs every chunk,
                # and the first half of the chunk's groups has both operands
                # (A0+B0) after 2 calls - matmuls start at ~50% of the gather.
                # idx stream stays laid out [A0, A1, B0, B1].
                t1 = half_t - t0
                splits = [(0, 0, 0, t0, 0),
                          (1, 1, 0, t0, hw),
                          (2, 0, t0, t1, t0 * 8),
                          (3, 1, t0, t1, hw + t0 * 8)]
                splits = [s for s in splits if s[3] > 0]
                idx_sb = bpool.tile([P, 2 * hw], i16, tag="idx")
                for q, side, lo, nt, off in splits:
                    nc.sync.dma_start(
                        idx_sb[32 * q:32 * q + 32, off:off + nt * 8],
                        idxt[:, cm["idx0"] + off:cm["idx0"] + off + nt * 8])
                ee_sb = bpool.tile([P, tiles * H], bf16, tag="ee")
                nc.sync.dma_start(ee_sb[:],
                                  eet[:, cm["ee0"]:cm["ee0"] + tiles * H])
                gx_sb = bpool.tile([P, tiles], bf16, tag="gx")
                nc.sync.dma_start(gx_sb[:], gxt[:, cm["gx0"]:cm["gx0"] + tiles])

                xga = gpool.tile([P, half_t * F], bf16, tag="xga")
                xgb = gpool.tile([P, half_t * F], bf16, tag="xgb")
                for q, side, lo, nt, off in splits:
                    sb = xgb if side else xga
                    nc.gpsimd.dma_gather(
                        out_ap=sb[:, lo * F:(lo + nt) * F]
                            .rearrange("p (k f) -> p k f", k=nt),
                        in_ap=xtab[HALF:2 * HALF] if side else xtab[0:HALF],
                        idxs_ap=idx_sb[:, off:off + nt * 8],
                        num_idxs=nt * P,
                        num_idxs_reg=nt * P,
                        elem_size=F,
                        single_packet=False,
                        queue_num=q,
                    )

                # mask[p, t, g] = (iota[g] == gidx[p, t]) for all tiles
                mask = wpool.tile([P, tiles * G], bf16, tag="mask")
                nc.vector.tensor_tensor(
                    out=mask[:].rearrange("p (t g) -> p t g", t=tiles),
                    in0=iota_sb[:].unsqueeze(1).broadcast_to([P, tiles, G]),
                    in1=gx_sb[:].unsqueeze(2).broadcast_to([P, tiles, G]),
                    op=mybir.AluOpType.is_equal,
                )
                # w[p, t, h, g] = mask[p, t, g] * ee[p, t, h]
                wgt = wpool.tile([P, tiles * C], bf16, tag="wgt")
                nc.vector.tensor_tensor(
                    out=wgt[:].rearrange("p (t h g) -> p t h g",
                                         t=tiles, h=H),
                    in0=mask[:].rearrange("p (t g) -> p t g", t=tiles)
                        .unsqueeze(2).broadcast_to([P, tiles, H, G]),
                    in1=ee_sb[:].rearrange("p (t h) -> p t h", t=tiles)
                        .unsqueeze(3).broadcast_to([P, tiles, H, G]),
                    op=mybir.AluOpType.mult,
                )

                # X-chunk group j: A tiles 2j,2j+1; B tiles half_t+2j(+1)
                # Y-chunk group j: A tile j; B tile half_t+j
                # group j: psum tile q=j//8, wq=j%8,
                #   partitions (wq%2)*64 + (h*16+col), free (wq//2)*128 + f
                pss = [pspool.tile([P, 512], f32, tag=f"ps{q}",
                                   name=f"ps{q}")
                       for q in range(nps)]
                for g in range(ng):
                    q, wq = g // 8, g % 8
                    po, fo = (wq % 2) * C, (wq // 2) * F
                    out_ap = pss[q][po:po + C, fo:fo + F]
                    if is_x:
                        tids = [2 * g, 2 * g + 1,
                                half_t + 2 * g, half_t + 2 * g + 1]
                        srcs = [xga, xga, xgb, xgb]
                        offs = [2 * g, 2 * g + 1, 2 * g, 2 * g + 1]
                    else:
                        tids = [g, half_t + g]
                        srcs = [xga, xgb]
                        offs = [g, g]
                    for j, (t, sb, o) in enumerate(zip(tids, srcs, offs)):
                        nc.tensor.matmul(
                            out=out_ap,
                            lhsT=wgt[:, t * C:(t + 1) * C],
                            rhs=sb[:, o * F:(o + 1) * F],
                            start=(j == 0), stop=(j == len(tids) - 1),
                        )

                ow = nps * 512
                osb = fpool.tile([P, ow], bf16, tag="osb")
                for q in range(nps):
                    nc.scalar.copy(osb[:, q * 512:(q + 1) * 512], pss[q][:])
                nc.sync.dma_start(outt[:, cm["out0"]:cm["out0"] + ow], osb[:])
    nc.compile()
    return nc


def _pack_device(a_d, b_d, target_ngx=None):
    """Two-pointer greedy packing over desc-degree-sorted nodes.

    Returns (gid, col, modes): node n -> group gid[n], column col[n];
    modes[k] in {0 (X: 2A+2B tiles), 1 (Y: 1A+1B)}; X groups first.
    If target_ngx is given, the X phase is stretched to that many groups
    (keeps per-device group counts aligned with the shared schedule).
    """
    deg = a_d + b_d
    order = np.argsort(-deg, kind="stable")
    a_s, b_s = a_d[order], b_d[order]
    n = len(a_s)
    wind = np.concatenate([np.cumsum(a_s + b_s), np.full(G, deg.sum())])

    gid_s = np.empty(n, np.int64)
    col_s = np.empty(n, np.int64)
    taken = np.zeros(n, bool)
    modes = []
    h, t = 0, n - 1
    y_phase = False
    k = 0
    while True:
        while h <= t and taken[h]:
            h += 1
        while t >= h and taken[t]:
            t -= 1
        if h > t:
            break
        if not y_phase:
            if target_ngx is not None:
                y_phase = k >= target_ngx
            else:
                lo = wind[h - 1] if h > 0 else 0
                if wind[min(h + G - 1, n - 1)] - lo < XSUM_MIN:
                    y_phase = True
        cap = P if y_phase else 2 * P
        remA, remB, cols = cap, cap, 0
        while h <= t and cols < G:
            if taken[h]:
                h += 1
                continue
            if a_s[h] <= remA and b_s[h] <= remB:
                gid_s[h] = k
                col_s[h] = cols
                remA -= a_s[h]
                remB -= b_s[h]
                cols += 1
                h += 1
            else:
                break
        # tail fill: bounded backward search over the smallest nodes for
        # anything that still fits the (remA, remB) leftovers
        j, scan = t, 0
        while j >= h and cols < G and scan < 384:
            if not taken[j] and a_s[j] <= remA and b_s[j] <= remB:
                gid_s[j] = k
                col_s[j] = cols
                remA -= a_s[j]
                remB -= b_s[j]
                cols += 1
                taken[j] = True
                if j == t:
                    t -= 1
            else:
                scan += 1
            j -= 1
        assert cols > 0
        modes.append(1 if y_phase else 0)
        k += 1
    gid = np.empty(n, np.int64)
    col = np.empty(n, np.int64)
    gid[order] = gid_s
    col[order] = col_s
    return gid, col, np.asarray(modes)


def kernel(x, w, a, edge_index):
    global _last_results
    x = np.asarray(x, dtype=np.float32)
    w = np.asarray(w, dtype=np.float32)
    a = np.asarray(a, dtype=np.float32)
    edge_index = np.asarray(edge_index)

    src = edge_index[0].astype(np.int64)
    dst = edge_index[1].astype(np.int64)

    # host: tiny projections + edge scores + ee (bf16, shared with rowsum)
    c_src = (w[:, 0, :] * a[:, :F, 0]).astype(np.float32)   # [H,F]
    c_dst = (w[:, 0, :] * a[:, F:, 0]).astype(np.float32)
    s_src = x @ c_src.T                                     # [N,H]
    s_dst = x @ c_dst.T
    score = s_src[src] + s_dst[dst]                         # [E,H]
    lk = np.where(score > 0, score, 0.2 * score)
    ee = np.exp(-lk, dtype=np.float32)                      # [E,H]
    ee_bf = ee.astype(ml_dtypes.bfloat16)
    ee64 = ee_bf.astype(np.float64)

    rs = np.zeros((H, N_PAD), np.float64)
    for h in range(H):
        rs[h] = np.bincount(src, weights=ee64[:, h], minlength=N_PAD)
    rs[rs == 0] = 1.0

    x_pad = np.zeros((N_PAD, F), np.float32)
    x_pad[:N_NODES] = x
    x_bf = np.ascontiguousarray(x_pad.astype(ml_dtypes.bfloat16))
    iota_np = np.broadcast_to(np.arange(G, dtype=np.float32), (P, G)
                              ).astype(ml_dtypes.bfloat16)

    isB = dst >= HALF
    degA = np.bincount(src, weights=~isB, minlength=N_PAD).astype(np.int64)
    degB = np.bincount(src, weights=isB, minlength=N_PAD).astype(np.int64)
    # a node whose per-side degree exceeds an X-group's side capacity could
    # never be placed; fail loudly instead of corrupting the packing
    assert degA.max() <= 2 * P and degB.max() <= 2 * P, \
        (degA.max(), degB.max())

    # ---- per-device packing ----
    # pass 1: natural X/Y split per device; pass 2: align every device to the
    # max X-group count so the shared schedule pads as little as possible
    ngx_nat = []
    for d in range(NDEV):
        lo = d * NPD
        _, _, modes = _pack_device(degA[lo:lo + NPD], degB[lo:lo + NPD])
        ngx_nat.append(int((modes == 0).sum()))
    ngx_tgt = max(ngx_nat)
    dev_pack = []
    ngx_max = ngy_max = 0
    for d in range(NDEV):
        lo = d * NPD
        gid, col, modes = _pack_device(degA[lo:lo + NPD], degB[lo:lo + NPD],
                                       target_ngx=ngx_tgt)
        ngx = int((modes == 0).sum())
        ngy = int((modes == 1).sum())
        ngx_max = max(ngx_max, ngx)
        ngy_max = max(ngy_max, ngy)
        dev_pack.append((gid, col, modes, ngx))

    # chunk schedule: full X-chunks of 16 groups (+ partial), then Y of 32
    chunks = []
    r = ngx_max
    while r > 0:
        chunks.append((True, min(16, r)))
        r -= min(16, r)
    r = ngy_max
    while r > 0:
        chunks.append((False, min(32, r)))
        r -= min(32, r)
    chunks = tuple(chunks)
    meta, ntiles, idx_cols, ee_cols, gx_cols, out_cols = _chunk_meta(chunks)
    # per-group (schedule-level) lookup tables
    sch_rows = []   # (is_x, chunk_idx, j_in_chunk)
    for ci, (is_x, ng) in enumerate(chunks):
        for j in range(ng):
            sch_rows.append((is_x, ci, j))
    sch_isx = np.array([r[0] for r in sch_rows])
    sch_ci = np.array([r[1] for r in sch_rows])
    sch_j = np.array([r[2] for r in sch_rows])
    m_tile0 = np.array([m["tile0"] for m in meta])
    m_half = np.array([m["half_t"] for m in meta])
    m_out0 = np.array([m["out0"] for m in meta])
    # X-group k (global order) must map to k-th X-slot of the schedule;
    # schedule lists X groups first, so global group id == schedule row.
    gA_base = np.where(sch_isx,
                       m_tile0[sch_ci] + 2 * sch_j,
                       m_tile0[sch_ci] + sch_j)
    gB_base = gA_base + m_half[sch_ci]
    g_out = m_out0[sch_ci] + (sch_j // 8) * 512 + ((sch_j % 8) // 2) * F
    g_prow = ((sch_j % 8) % 2) * C

    in_maps = []
    for d in range(NDEV):
        lo = d * NPD
        gid, col, modes, ngx = dev_pack[d]
        # device group id -> schedule row: X groups k -> k; Y groups k -> (k -
        # ngx) + ngx_max
        m = (src >= lo) & (src < lo + NPD)
        es = src[m] - lo
        ed = dst[m]
        eb = isB[m]
        eee = ee_bf[m]                       # [Ed, H] bf16
        sgid = np.where(modes == 0, np.arange(len(modes)),
                        np.arange(len(modes)) - ngx + ngx_max)
        egid = sgid[gid[es]]
        ecol = col[es]

        # rank of each edge within its (group, side) bucket
        okey = np.lexsort((eb, egid))
        ed_o, eb_o = ed[okey], eb[okey]
        egid_o, ecol_o = egid[okey], ecol[okey]
        eee_o = eee[okey]
        bucket = egid_o * 2 + eb_o
        bchange = np.flatnonzero(np.diff(bucket)) + 1
        starts = np.concatenate([[0], bchange])
        bid = np.zeros(len(bucket), np.int64)
        bid[bchange] = 1
        bid = np.cumsum(bid)
        rank = np.arange(len(bucket)) - starts[bid]

        base = np.where(eb_o, gB_base[egid_o], gA_base[egid_o])
        tt = base + (rank >> 7)
        p = rank & 127

        idx_flat = np.zeros((ntiles, P), np.int16)  # [tile, slot]
        idx_flat[tt, p] = (ed_o - np.where(eb_o, HALF, 0)).astype(np.int16)

        # -1 for the strictly-trailing pad of each gather call: the Q7 trims
        # trailing negatives, skipping those descriptors entirely.
        # wrap per gather call ([A0, A1, B0, B1] per chunk, matching the
        # device's 4-way split): flat (tile, slot) -> [i%16, i//16] x2 bands
        idx_cols_dev = np.empty((32, idx_cols), np.int16)
        colpos = 0
        for cm in meta:
            t0s = (cm["half_t"] + 1) // 2
            for side in range(2):
                base = cm["tile0"] + side * cm["half_t"]
                for lo, nt in ((0, t0s), (t0s, cm["half_t"] - t0s)):
                    if not nt:
                        continue
                    fl = idx_flat[base + lo:base + lo + nt].reshape(-1)
                    wq = fl.reshape(-1, 16).T    # [16, nt*8]
                    idx_cols_dev[:, colpos:colpos + nt * 8] = \
                        np.tile(wq, (2, 1))
                    colpos += nt * 8
        assert colpos == idx_cols

        ee_arr = np.zeros((P, ee_cols), ml_dtypes.bfloat16)
        ee_arr[p[:, None], (tt * H)[:, None] + np.arange(H)[None, :]] = eee_o
        gx_arr = np.zeros((P, gx_cols), np.float32)
        gx_arr[p, tt] = ecol_o.astype(np.float32)

        in_maps.append({
            "xtab": x_bf,
            "idxt": idx_cols_dev,
            "eet": np.ascontiguousarray(ee_arr),
            "gxt": gx_arr.astype(ml_dtypes.bfloat16),
            "iota": iota_np,
        })

    if chunks not in _program_cache:
        _program_cache[chunks] = _build_program(chunks)
    nc = _program_cache[chunks]

    res = run_bass_kernel_spmd(nc, in_maps, core_ids=list(range(NDEV)))
    _last_results = res

    # ---- decode ----
    out = np.empty((H, N_PAD, F), np.float32)
    hh = np.arange(H)
    for d in range(NDEV):
        gid, col, modes, ngx = dev_pack[d]
        r = res.results[d]["outt"].astype(np.float32)   # [P, out_cols]
        sgid = np.where(modes == 0, np.arange(len(modes)),
                        np.arange(len(modes)) - ngx + ngx_max)
        node_s = sgid[gid]                              # schedule row per node
        pn = g_prow[node_s] + col                       # h=0 partition row
        fn = g_out[node_s]                              # col base
        idx_p = pn[None, :, None] + (hh * G)[:, None, None]
        idx_f = fn[None, :, None] + np.arange(F)[None, None, :]
        out[:, d * NPD:(d + 1) * NPD, :] = \
            r[np.broadcast_to(idx_p, (H, NPD, F)),
              np.broadcast_to(idx_f, (H, NPD, F))]
    out *= w[:, 0, :][:, None, :]
    out /= rs[:, :, None].astype(np.float32)
    return np.ascontiguousarray(out[:, :N_NODES, :]).astype(np.float32)


# revision 14
# speedup vs baseline: 1.0166x; 1.0031x over previous
"""MultiHeadGraphAttention kernel for 8 Trainium2 NeuronCores — v2.

Strategy (vs the v1 staged kernel): eliminate the per-tile DVE wall-build
bottleneck by making the PE consume a block-diagonal attention-weight matrix
(lhsT) that is assembled on-device with two batched DVE ops per chunk.

Sharding: device d owns src nodes [d*6256, (d+1)*6256) and ALL their edges
(~200K). Host packs nodes into "groups" of <=16 nodes; a group's A-edges
(dst < 25024) and B-edges (dst >= 25024) each fill 2 tiles (X-groups, for
high-degree nodes) or 1 tile (Y-groups) of 128 slots. Per group the PE runs
accumulating matmuls:
    psum[64 cols (4h x 16 nodes), 128 f] += W_tile.T @ Xg_tile
where W_tile[slot, h*16+g] = (gidx[slot]==g) * ee[h, slot] is built by DVE
(is_equal + mult with broadcast access patterns) and Xg is dma_gather'ed
(int16 idx; two base pointers into one table dodge the 32767 limit; gather
descriptor generation is spread over all 4 SWDGE queues / Q7 core pairs;
pad slots gather row 0 and are zeroed by ee=0).
Host precomputes ee = exp(-leaky_relu(score)) in bf16 and divides by the
rowsums / applies the diag weight w after gathering device outputs.
"""

import sys

sys.path.insert(0, "/opt/trn_rl_repo")

import ml_dtypes
import numpy as np

import concourse.bass as bass
import concourse.tile as tile
from concourse import bacc, mybir
from concourse.bass_utils import run_bass_kernel_spmd
from concourse.library_config import mlp

N_NODES = 50000
N_PAD = 50048
NDEV = 8
NPD = N_PAD // NDEV          # 6256 src nodes per device
HALF = 25024                 # A: dst in [0,25024), B: [25024,50048)
P = 128
F = 128
H = 4
G = 16                       # node columns per group
C = H * G                    # 64 psum cols per group
XSUM_MIN = 480               # switch X->Y when next 16 nodes sum below this

_last_results = None
_program_cache = {}
# Trailing-(-1) idx trimming is UNSAFE: the NRT decode side reserves DMA ring
# space from the static num_idxs ("keep in lockstep" in dma_gather.hpp); a Q7
# that pushes fewer descriptors desyncs the rings -> wild DMA -> device hang.
_TRIM_TRAILING = False


def _chunk_meta(chunks):
    """Per-chunk derived offsets. chunks: tuple of (is_x, ngroups)."""
    meta = []
    tile0 = idx0 = eg0 = out0 = 0
    for is_x, ng in chunks:
        tiles = (4 if is_x else 2) * ng
        half_t = tiles // 2
        nps = -(-ng // 8)
        meta.append(dict(is_x=is_x, ng=ng, tiles=tiles, half_t=half_t,
                         nps=nps, tile0=tile0, idx0=idx0, eg0=eg0,
                         out0=out0))
        tile0 += tiles
        idx0 += tiles * 8          # idx cols (i16): tiles*128/16
        eg0 += tiles * 5           # per tile: 4 ee cols + 1 gidx col (bf16)
        out0 += nps * 512
    return meta, tile0, idx0, eg0, out0


def _build_program(chunks):
    f32 = mybir.dt.float32
    bf16 = mybir.dt.bfloat16
    i16 = mybir.dt.int16
    meta, ntiles, idx_cols, eg_cols, out_cols = _chunk_meta(chunks)

    nc = bacc.Bacc("TRN2", target_bir_lowering=False, debug=False,
                   num_devices=NDEV, num_swdge_queues=4)

    xtab = nc.dram_tensor("xtab", [N_PAD, F], bf16, kind="ExternalInput").ap()
    idxt = nc.dram_tensor("idxt", [32, idx_cols], i16,
                          kind="ExternalInput").ap()
    egt = nc.dram_tensor("egt", [P, eg_cols], bf16, kind="ExternalInput").ap()
    iota = nc.dram_tensor("iota", [P, G], bf16, kind="ExternalInput").ap()
    outt = nc.dram_tensor("outt", [P, out_cols], bf16,
                          kind="ExternalOutput").ap()

    with tile.TileContext(nc) as tc:
        with (
            tc.tile_pool(name="const", bufs=1) as cpool,
            tc.tile_pool(name="blkin", bufs=6) as bpool,
            tc.tile_pool(name="gath", bufs=4) as gpool,
            tc.tile_pool(name="wbuf", bufs=3) as wpool,
            tc.tile_pool(name="fin", bufs=3) as fpool,
            tc.tile_pool(name="psum", bufs=2, space="PSUM") as pspool,
        ):
            nc.gpsimd.load_library(mlp)
            iota_sb = cpool.tile([P, G], bf16)
            nc.sync.dma_start(iota_sb[:], iota[:, :])

            for c, cm in enumerate(meta):
                is_x, ng, tiles = cm["is_x"], cm["ng"], cm["tiles"]
                half_t, nps = cm["half_t"], cm["nps"]
                hw = half_t * 8    # idx cols per side
                t0 = (half_t + 1) // 2
                # 4 half-gathers per chunk issued as A0,B0,A1,B1 on queues
                # 0-3: spreads Q7 desc-gen over all 4 core pairs every chunk,
                # and the first half of the chunk's groups has both operands
                # (A0+B0) after 2 calls - matmuls start at ~50% of the gather.
                # idx stream stays laid out [A0, A1, B0, B1].
                t1 = half_t - t0
                splits = [(0, 0, 0, t0, 0),
                          (1, 1, 0, t0, hw),
                          (2, 0, t0, t1, t0 * 8),
                          (3, 1, t0, t1, hw + t0 * 8)]
                splits = [s for s in splits if s[3] > 0]
                idx_sb = bpool.tile([P, 2 * hw], i16, tag="idx")
                for q, side, lo, nt, off in splits:
                    nc.sync.dma_start(
                        idx_sb[32 * q:32 * q + 32, off:off + nt * 8],
                        idxt[:, cm["idx0"] + off:cm["idx0"] + off + nt * 8])
                eg_sb = bpool.tile([P, tiles * 5], bf16, tag="eg")
                nc.sync.dma_start(eg_sb[:],
                                  egt[:, cm["eg0"]:cm["eg0"] + tiles * 5])
                eg3 = eg_sb[:].rearrange("p (t c) -> p t c", c=5)

                xga = gpool.tile([P, half_t * F], bf16, tag="xga")
                xgb = gpool.tile([P, half_t * F], bf16, tag="xgb")
                for q, side, lo, nt, off in splits:
                    sb = xgb if side else xga
                    nc.gpsimd.dma_gather(
                        out_ap=sb[:, lo * F:(lo + nt) * F]
                            .rearrange("p (k f) -> p k f", k=nt),
                        in_ap=xtab[HALF:2 * HALF] if side else xtab[0:HALF],
                        idxs_ap=idx_sb[:, off:off + nt * 8],
                        num_idxs=nt * P,
                        num_idxs_reg=nt * P,
                        elem_size=F,
                        single_packet=False,
                        queue_num=q,
                    )

                # mask[p, t, g] = (iota[g] == gidx[p, t]) for all tiles
                mask = wpool.tile([P, tiles * G], bf16, tag="mask")
                nc.vector.tensor_tensor(
                    out=mask[:].rearrange("p (t g) -> p t g", t=tiles),
                    in0=iota_sb[:].unsqueeze(1).broadcast_to([P, tiles, G]),
                    in1=eg3[:, :, 4:5].broadcast_to([P, tiles, G]),
                    op=mybir.AluOpType.is_equal,
                )
                # w[p, t, h, g] = mask[p, t, g] * ee[p, t, h]
                wgt = wpool.tile([P, tiles * C], bf16, tag="wgt")
                nc.vector.tensor_tensor(
                    out=wgt[:].rearrange("p (t h g) -> p t h g",
                                         t=tiles, h=H),
                    in0=mask[:].rearrange("p (t g) -> p t g", t=tiles)
                        .unsqueeze(2).broadcast_to([P, tiles, H, G]),
                    in1=eg3[:, :, 0:4].unsqueeze(3)
                        .broadcast_to([P, tiles, H, G]),
                    op=mybir.AluOpType.mult,
                )

                # X-chunk group j: A tiles 2j,2j+1; B tiles half_t+2j(+1)
                # Y-chunk group j: A tile j; B tile half_t+j
                # group j: psum tile q=j//8, wq=j%8,
                #   partitions (wq%2)*64 + (h*16+col), free (wq//2)*128 + f
                pss = [pspool.tile([P, 512], f32, tag=f"ps{q}",
                                   name=f"ps{q}")
                       for q in range(nps)]
                for g in range(ng):
                    q, wq = g // 8, g % 8
                    po, fo = (wq % 2) * C, (wq // 2) * F
                    out_ap = pss[q][po:po + C, fo:fo + F]
                    if is_x:
                        tids = [2 * g, 2 * g + 1,
                                half_t + 2 * g, half_t + 2 * g + 1]
                        srcs = [xga, xga, xgb, xgb]
                        offs = [2 * g, 2 * g + 1, 2 * g, 2 * g + 1]
                    else:
                        tids = [g, half_t + g]
                        srcs = [xga, xgb]
                        offs = [g, g]
                    for j, (t, sb, o) in enumerate(zip(tids, srcs, offs)):
                        nc.tensor.matmul(
                            out=out_ap,
                            lhsT=wgt[:, t * C:(t + 1) * C],
                            rhs=sb[:, o * F:(o + 1) * F],
                            start=(j == 0), stop=(j == len(tids) - 1),
                        )

                ow = nps * 512
                osb = fpool.tile([P, ow], bf16, tag="osb")
                for q in range(nps):
                    nc.scalar.copy(osb[:, q * 512:(q + 1) * 512], pss[q][:])
                nc.sync.dma_start(outt[:, cm["out0"]:cm["out0"] + ow], osb[:])
    nc.compile()
    return nc


def _pack_device(a_d, b_d, target_ngx=None):
    """Two-pointer greedy packing over desc-degree-sorted nodes.

    Returns (gid, col, modes): node n -> group gid[n], column col[n];
    modes[k] in {0 (X: 2A+2B tiles), 1 (Y: 1A+1B)}; X groups first.
    If target_ngx is given, the X phase is stretched to that many groups
    (keeps per-device group counts aligned with the shared schedule).
    """
    deg = a_d + b_d
    order = np.argsort(-deg, kind="stable")
    a_s, b_s = a_d[order], b_d[order]
    n = len(a_s)
    wind = np.concatenate([np.cumsum(a_s + b_s), np.full(G, deg.sum())])

    gid_s = np.empty(n, np.int64)
    col_s = np.empty(n, np.int64)
    taken = np.zeros(n, bool)
    modes = []
    h, t = 0, n - 1
    y_phase = False
    k = 0
    while True:
        while h <= t and taken[h]:
            h += 1
        while t >= h and taken[t]:
            t -= 1
        if h > t:
            break
        if not y_phase:
            if target_ngx is not None:
                y_phase = k >= target_ngx
            else:
                lo = wind[h - 1] if h > 0 else 0
                if wind[min(h + G - 1, n - 1)] - lo < XSUM_MIN:
                    y_phase = True
        cap = P if y_phase else 2 * P
        remA, remB, cols = cap, cap, 0
        while h <= t and cols < G:
            if taken[h]:
                h += 1
                continue
            if a_s[h] <= remA and b_s[h] <= remB:
                gid_s[h] = k
                col_s[h] = cols
                remA -= a_s[h]
                remB -= b_s[h]
                cols += 1
                h += 1
            else:
                break
        # tail fill: bounded backward search over the smallest nodes for
        # anything that still fits the (remA, remB) leftovers
        j, scan = t, 0
        while j >= h and cols < G and scan < 384:
            if not taken[j] and a_s[j] <= remA and b_s[j] <= remB:
                gid_s[j] = k
                col_s[j] = cols
                remA -= a_s[j]
                remB -= b_s[j]
                cols += 1
                taken[j] = True
                if j == t:
                    t -= 1
            else:
                scan += 1
            j -= 1
        assert cols > 0
        modes.append(1 if y_phase else 0)
        k += 1
    gid = np.empty(n, np.int64)
    col = np.empty(n, np.int64)
    gid[order] = gid_s
    col[order] = col_s
    return gid, col, np.asarray(modes)


def kernel(x, w, a, edge_index):
    global _last_results
    x = np.asarray(x, dtype=np.float32)
    w = np.asarray(w, dtype=np.float32)
    a = np.asarray(a, dtype=np.float32)
    edge_index = np.asarray(edge_index)

    src = edge_index[0].astype(np.int64)
    dst = edge_index[1].astype(np.int64)

    # host: tiny projections + edge scores + ee (bf16, shared with rowsum)
    c_src = (w[:, 0, :] * a[:, :F, 0]).astype(np.float32)   # [H,F]
    c_dst = (w[:, 0, :] * a[:, F:, 0]).astype(np.float32)
    s_src = x @ c_src.T                                     # [N,H]
    s_dst = x @ c_dst.T
    score = s_src[src] + s_dst[dst]                         # [E,H]
    lk = np.where(score > 0, score, 0.2 * score)
    ee = np.exp(-lk, dtype=np.float32)                      # [E,H]
    ee_bf = ee.astype(ml_dtypes.bfloat16)
    ee64 = ee_bf.astype(np.float64)

    rs = np.zeros((H, N_PAD), np.float64)
    for h in range(H):
        rs[h] = np.bincount(src, weights=ee64[:, h], minlength=N_PAD)
    rs[rs == 0] = 1.0

    x_pad = np.zeros((N_PAD, F), np.float32)
    x_pad[:N_NODES] = x
    x_bf = np.ascontiguousarray(x_pad.astype(ml_dtypes.bfloat16))
    iota_np = np.broadcast_to(np.arange(G, dtype=np.float32), (P, G)
                              ).astype(ml_dtypes.bfloat16)

    isB = dst >= HALF
    degA = np.bincount(src, weights=~isB, minlength=N_PAD).astype(np.int64)
    degB = np.bincount(src, weights=isB, minlength=N_PAD).astype(np.int64)
    # a node whose per-side degree exceeds an X-group's side capacity could
    # never be placed; fail loudly instead of corrupting the packing
    assert degA.max() <= 2 * P and degB.max() <= 2 * P, \
        (degA.max(), degB.max())

    # ---- per-device packing ----
    # pass 1: natural X/Y split per device; pass 2: align every device to the
    # max X-group count so the shared schedule pads as little as possible
    ngx_nat = []
    for d in range(NDEV):
        lo = d * NPD
        _, _, modes = _pack_device(degA[lo:lo + NPD], degB[lo:lo + NPD])
        ngx_nat.append(int((modes == 0).sum()))
    ngx_tgt = max(ngx_nat)
    dev_pack = []
    ngx_max = ngy_max = 0
    for d in range(NDEV):
        lo = d * NPD
        gid, col, modes = _pack_device(degA[lo:lo + NPD], degB[lo:lo + NPD],
                                       target_ngx=ngx_tgt)
        ngx = int((modes == 0).sum())
        ngy = int((modes == 1).sum())
        ngx_max = max(ngx_max, ngx)
        ngy_max = max(ngy_max, ngy)
        dev_pack.append((gid, col, modes, ngx))

    # chunk schedule: full X-chunks of 16 groups (+ partial), then Y of 32
    chunks = []
    r = ngx_max
    while r > 0:
        chunks.append((True, min(16, r)))
        r -= min(16, r)
    r = ngy_max
    while r > 0:
        chunks.append((False, min(32, r)))
        r -= min(32, r)
    chunks = tuple(chunks)
    meta, ntiles, idx_cols, eg_cols, out_cols = _chunk_meta(chunks)
    # per-group (schedule-level) lookup tables
    sch_rows = []   # (is_x, chunk_idx, j_in_chunk)
    for ci, (is_x, ng) in enumerate(chunks):
        for j in range(ng):
            sch_rows.append((is_x, ci, j))
    sch_isx = np.array([r[0] for r in sch_rows])
    sch_ci = np.array([r[1] for r in sch_rows])
    sch_j = np.array([r[2] for r in sch_rows])
    m_tile0 = np.array([m["tile0"] for m in meta])
    m_half = np.array([m["half_t"] for m in meta])
    m_out0 = np.array([m["out0"] for m in meta])
    # X-group k (global order) must map to k-th X-slot of the schedule;
    # schedule lists X groups first, so global group id == schedule row.
    gA_base = np.where(sch_isx,
                       m_tile0[sch_ci] + 2 * sch_j,
                       m_tile0[sch_ci] + sch_j)
    gB_base = gA_base + m_half[sch_ci]
    g_out = m_out0[sch_ci] + (sch_j // 8) * 512 + ((sch_j % 8) // 2) * F
    g_prow = ((sch_j % 8) % 2) * C

    in_maps = []
    for d in range(NDEV):
        lo = d * NPD
        gid, col, modes, ngx = dev_pack[d]
        # device group id -> schedule row: X groups k -> k; Y groups k -> (k -
        # ngx) + ngx_max
        m = (src >= lo) & (src < lo + NPD)
        es = src[m] - lo
        ed = dst[m]
        eb = isB[m]
        eee = ee_bf[m]                       # [Ed, H] bf16
        sgid = np.where(modes == 0, np.arange(len(modes)),
                        np.arange(len(modes)) - ngx + ngx_max)
        egid = sgid[gid[es]]
        ecol = col[es]

        # rank of each edge within its (group, side) bucket
        okey = np.lexsort((eb, egid))
        ed_o, eb_o = ed[okey], eb[okey]
        egid_o, ecol_o = egid[okey], ecol[okey]
        eee_o = eee[okey]
        bucket = egid_o * 2 + eb_o
        bchange = np.flatnonzero(np.diff(bucket)) + 1
        starts = np.concatenate([[0], bchange])
        bid = np.zeros(len(bucket), np.int64)
        bid[bchange] = 1
        bid = np.cumsum(bid)
        rank = np.arange(len(bucket)) - starts[bid]

        base = np.where(eb_o, gB_base[egid_o], gA_base[egid_o])
        tt = base + (rank >> 7)
        p = rank & 127

        idx_flat = np.zeros((ntiles, P), np.int16)  # [tile, slot]
        idx_flat[tt, p] = (ed_o - np.where(eb_o, HALF, 0)).astype(np.int16)

        # -1 for the strictly-trailing pad of each gather call: the Q7 trims
        # trailing negatives, skipping those descriptors entirely.
        # wrap per gather call ([A0, A1, B0, B1] per chunk, matching the
        # device's 4-way split): flat (tile, slot) -> [i%16, i//16] x2 bands
        idx_cols_dev = np.empty((32, idx_cols), np.int16)
        colpos = 0
        for cm in meta:
            t0s = (cm["half_t"] + 1) // 2
            for side in range(2):
                base = cm["tile0"] + side * cm["half_t"]
                for lo, nt in ((0, t0s), (t0s, cm["half_t"] - t0s)):
                    if not nt:
                        continue
                    fl = idx_flat[base + lo:base + lo + nt].reshape(-1)
                    wq = fl.reshape(-1, 16).T    # [16, nt*8]
                    idx_cols_dev[:, colpos:colpos + nt * 8] = \
                        np.tile(wq, (2, 1))
                    colpos += nt * 8
        assert colpos == idx_cols

        eg_arr = np.zeros((P, eg_cols), ml_dtypes.bfloat16)
        eg_arr[p[:, None], (tt * 5)[:, None] + np.arange(H)[None, :]] = eee_o
        eg_arr[p, tt * 5 + 4] = ecol_o.astype(np.float32).astype(
            ml_dtypes.bfloat16)

        in_maps.append({
            "xtab": x_bf,
            "idxt": idx_cols_dev,
            "egt": np.ascontiguousarray(eg_arr),
            "iota": iota_np,
        })

    if chunks not in _program_cache:
        _program_cache[chunks] = _build_program(chunks)
    nc = _program_cache[chunks]

    res = run_bass_kernel_spmd(nc, in_maps, core_ids=list(range(NDEV)))
    _last_results = res

    # ---- decode ----
    out = np.empty((H, N_PAD, F), np.float32)
    hh = np.arange(H)
    for d in range(NDEV):
        gid, col, modes, ngx = dev_pack[d]
        r = res.results[d]["outt"].astype(np.float32)   # [P, out_cols]
        sgid = np.where(modes == 0, np.arange(len(modes)),
                        np.arange(len(modes)) - ngx + ngx_max)
        node_s = sgid[gid]                              # schedule row per node
        pn = g_prow[node_s] + col                       # h=0 partition row
        fn = g_out[node_s]                              # col base
        idx_p = pn[None, :, None] + (hh * G)[:, None, None]
        idx_f = fn[None, :, None] + np.arange(F)[None, None, :]
        out[:, d * NPD:(d + 1) * NPD, :] = \
            r[np.broadcast_to(idx_p, (H, NPD, F)),
              np.broadcast_to(idx_f, (H, NPD, F))]
    out *= w[:, 0, :][:, None, :]
    out /= rs[:, :, None].astype(np.float32)
    return np.ascontiguousarray(out[:, :N_NODES, :]).astype(np.float32)


# revision 15
# speedup vs baseline: 1.0179x; 1.0013x over previous
"""MultiHeadGraphAttention kernel for 8 Trainium2 NeuronCores — v2.

Strategy (vs the v1 staged kernel): eliminate the per-tile DVE wall-build
bottleneck by making the PE consume a block-diagonal attention-weight matrix
(lhsT) that is assembled on-device with two batched DVE ops per chunk.

Sharding: device d owns src nodes [d*6256, (d+1)*6256) and ALL their edges
(~200K). Host packs nodes into "groups" of <=16 nodes; a group's A-edges
(dst < 25024) and B-edges (dst >= 25024) each fill 2 tiles (X-groups, for
high-degree nodes) or 1 tile (Y-groups) of 128 slots. Per group the PE runs
accumulating matmuls:
    psum[64 cols (4h x 16 nodes), 128 f] += W_tile.T @ Xg_tile
where W_tile[slot, h*16+g] = (gidx[slot]==g) * ee[h, slot] is built by DVE
(is_equal + mult with broadcast access patterns) and Xg is dma_gather'ed
(int16 idx; two base pointers into one table dodge the 32767 limit; gather
descriptor generation is spread over all 4 SWDGE queues / Q7 core pairs;
pad slots gather row 0 and are zeroed by ee=0).
Host precomputes ee = exp(-leaky_relu(score)) in bf16 and divides by the
rowsums / applies the diag weight w after gathering device outputs.
"""

import sys

sys.path.insert(0, "/opt/trn_rl_repo")

import ml_dtypes
import numpy as np

import concourse.bass as bass
import concourse.tile as tile
from concourse import bacc, mybir
from concourse.bass_utils import run_bass_kernel_spmd
from concourse.library_config import mlp

N_NODES = 50000
N_PAD = 50048
NDEV = 8
NPD = N_PAD // NDEV          # 6256 src nodes per device
HALF = 25024                 # A: dst in [0,25024), B: [25024,50048)
P = 128
F = 128
H = 4
G = 16                       # node columns per group
C = H * G                    # 64 psum cols per group
XSUM_MIN = 480               # switch X->Y when next 16 nodes sum below this

_last_results = None
_program_cache = {}
# Trailing-(-1) idx trimming is UNSAFE: the NRT decode side reserves DMA ring
# space from the static num_idxs ("keep in lockstep" in dma_gather.hpp); a Q7
# that pushes fewer descriptors desyncs the rings -> wild DMA -> device hang.
_TRIM_TRAILING = False


def _chunk_meta(chunks):
    """Per-chunk derived offsets. chunks: tuple of (is_x, ngroups)."""
    meta = []
    tile0 = idx0 = eg0 = out0 = 0
    for is_x, ng in chunks:
        tiles = (4 if is_x else 2) * ng
        half_t = tiles // 2
        nps = -(-ng // 8)
        meta.append(dict(is_x=is_x, ng=ng, tiles=tiles, half_t=half_t,
                         nps=nps, tile0=tile0, idx0=idx0, eg0=eg0,
                         out0=out0))
        tile0 += tiles
        idx0 += ((half_t + 1) // 2) * 8   # idx cols: one t0-wide block,
                                          # 4 queue bands stacked in rows
        eg0 += tiles * 5           # per tile: 4 ee cols + 1 gidx col (bf16)
        out0 += nps * 512
    return meta, tile0, idx0, eg0, out0


def _build_program(chunks):
    f32 = mybir.dt.float32
    bf16 = mybir.dt.bfloat16
    i16 = mybir.dt.int16
    meta, ntiles, idx_cols, eg_cols, out_cols = _chunk_meta(chunks)

    nc = bacc.Bacc("TRN2", target_bir_lowering=False, debug=False,
                   num_devices=NDEV, num_swdge_queues=4)

    xtab = nc.dram_tensor("xtab", [N_PAD, F], bf16, kind="ExternalInput").ap()
    idxt = nc.dram_tensor("idxt", [P, idx_cols], i16,
                          kind="ExternalInput").ap()
    egt = nc.dram_tensor("egt", [P, eg_cols], bf16, kind="ExternalInput").ap()
    iota = nc.dram_tensor("iota", [P, G], bf16, kind="ExternalInput").ap()
    outt = nc.dram_tensor("outt", [P, out_cols], bf16,
                          kind="ExternalOutput").ap()

    with tile.TileContext(nc) as tc:
        with (
            tc.tile_pool(name="const", bufs=1) as cpool,
            tc.tile_pool(name="blkin", bufs=6) as bpool,
            tc.tile_pool(name="gath", bufs=4) as gpool,
            tc.tile_pool(name="wbuf", bufs=3) as wpool,
            tc.tile_pool(name="fin", bufs=3) as fpool,
            tc.tile_pool(name="psum", bufs=2, space="PSUM") as pspool,
        ):
            nc.gpsimd.load_library(mlp)
            iota_sb = cpool.tile([P, G], bf16)
            nc.sync.dma_start(iota_sb[:], iota[:, :])

            for c, cm in enumerate(meta):
                is_x, ng, tiles = cm["is_x"], cm["ng"], cm["tiles"]
                half_t, nps = cm["half_t"], cm["nps"]
                t0 = (half_t + 1) // 2
                # 4 half-gathers per chunk issued as A0,B0,A1,B1 on queues
                # 0-3: spreads Q7 desc-gen over all 4 core pairs every chunk,
                # and the first half of the chunk's groups has both operands
                # (A0+B0) after 2 calls - matmuls start at ~50% of the gather.
                # Queue q's Q7 pair reads idx data from partition band
                # [32q, 32q+32), so all 4 calls share one idx column block:
                # rows = [A0 | B0 | A1 | B1] wraps, one DMA per chunk.
                t1 = half_t - t0
                splits = [(0, 0, 0, t0),
                          (1, 1, 0, t0),
                          (2, 0, t0, t1),
                          (3, 1, t0, t1)]
                splits = [s for s in splits if s[3] > 0]
                idx_sb = bpool.tile([P, t0 * 8], i16, tag="idx")
                nc.sync.dma_start(idx_sb[:],
                                  idxt[:, cm["idx0"]:cm["idx0"] + t0 * 8])
                eg_sb = bpool.tile([P, tiles * 5], bf16, tag="eg")
                nc.sync.dma_start(eg_sb[:],
                                  egt[:, cm["eg0"]:cm["eg0"] + tiles * 5])
                eg3 = eg_sb[:].rearrange("p (t c) -> p t c", c=5)

                xga = gpool.tile([P, half_t * F], bf16, tag="xga")
                xgb = gpool.tile([P, half_t * F], bf16, tag="xgb")
                for q, side, lo, nt in splits:
                    sb = xgb if side else xga
                    nc.gpsimd.dma_gather(
                        out_ap=sb[:, lo * F:(lo + nt) * F]
                            .rearrange("p (k f) -> p k f", k=nt),
                        in_ap=xtab[HALF:2 * HALF] if side else xtab[0:HALF],
                        idxs_ap=idx_sb[:, 0:nt * 8],
                        num_idxs=nt * P,
                        num_idxs_reg=nt * P,
                        elem_size=F,
                        single_packet=False,
                        queue_num=q,
                    )

                # mask[p, t, g] = (iota[g] == gidx[p, t]) for all tiles
                mask = wpool.tile([P, tiles * G], bf16, tag="mask")
                nc.vector.tensor_tensor(
                    out=mask[:].rearrange("p (t g) -> p t g", t=tiles),
                    in0=iota_sb[:].unsqueeze(1).broadcast_to([P, tiles, G]),
                    in1=eg3[:, :, 4:5].broadcast_to([P, tiles, G]),
                    op=mybir.AluOpType.is_equal,
                )
                # w[p, t, h, g] = mask[p, t, g] * ee[p, t, h]
                wgt = wpool.tile([P, tiles * C], bf16, tag="wgt")
                nc.vector.tensor_tensor(
                    out=wgt[:].rearrange("p (t h g) -> p t h g",
                                         t=tiles, h=H),
                    in0=mask[:].rearrange("p (t g) -> p t g", t=tiles)
                        .unsqueeze(2).broadcast_to([P, tiles, H, G]),
                    in1=eg3[:, :, 0:4].unsqueeze(3)
                        .broadcast_to([P, tiles, H, G]),
                    op=mybir.AluOpType.mult,
                )

                # X-chunk group j: A tiles 2j,2j+1; B tiles half_t+2j(+1)
                # Y-chunk group j: A tile j; B tile half_t+j
                # group j: psum tile q=j//8, wq=j%8,
                #   partitions (wq%2)*64 + (h*16+col), free (wq//2)*128 + f
                pss = [pspool.tile([P, 512], f32, tag=f"ps{q}",
                                   name=f"ps{q}")
                       for q in range(nps)]
                for g in range(ng):
                    q, wq = g // 8, g % 8
                    po, fo = (wq % 2) * C, (wq // 2) * F
                    out_ap = pss[q][po:po + C, fo:fo + F]
                    if is_x:
                        tids = [2 * g, 2 * g + 1,
                                half_t + 2 * g, half_t + 2 * g + 1]
                        srcs = [xga, xga, xgb, xgb]
                        offs = [2 * g, 2 * g + 1, 2 * g, 2 * g + 1]
                    else:
                        tids = [g, half_t + g]
                        srcs = [xga, xgb]
                        offs = [g, g]
                    for j, (t, sb, o) in enumerate(zip(tids, srcs, offs)):
                        nc.tensor.matmul(
                            out=out_ap,
                            lhsT=wgt[:, t * C:(t + 1) * C],
                            rhs=sb[:, o * F:(o + 1) * F],
                            start=(j == 0), stop=(j == len(tids) - 1),
                        )

                ow = nps * 512
                osb = fpool.tile([P, ow], bf16, tag="osb")
                for q in range(nps):
                    nc.scalar.copy(osb[:, q * 512:(q + 1) * 512], pss[q][:])
                nc.sync.dma_start(outt[:, cm["out0"]:cm["out0"] + ow], osb[:])
    nc.compile()
    return nc


def _pack_device(a_d, b_d, target_ngx=None):
    """Two-pointer greedy packing over desc-degree-sorted nodes.

    Returns (gid, col, modes): node n -> group gid[n], column col[n];
    modes[k] in {0 (X: 2A+2B tiles), 1 (Y: 1A+1B)}; X groups first.
    If target_ngx is given, the X phase is stretched to that many groups
    (keeps per-device group counts aligned with the shared schedule).
    """
    deg = a_d + b_d
    order = np.argsort(-deg, kind="stable")
    a_s, b_s = a_d[order], b_d[order]
    n = len(a_s)
    wind = np.concatenate([np.cumsum(a_s + b_s), np.full(G, deg.sum())])

    gid_s = np.empty(n, np.int64)
    col_s = np.empty(n, np.int64)
    taken = np.zeros(n, bool)
    modes = []
    h, t = 0, n - 1
    y_phase = False
    k = 0
    while True:
        while h <= t and taken[h]:
            h += 1
        while t >= h and taken[t]:
            t -= 1
        if h > t:
            break
        if not y_phase:
            if target_ngx is not None:
                y_phase = k >= target_ngx
            else:
                lo = wind[h - 1] if h > 0 else 0
                if wind[min(h + G - 1, n - 1)] - lo < XSUM_MIN:
                    y_phase = True
        cap = P if y_phase else 2 * P
        remA, remB, cols = cap, cap, 0
        while h <= t and cols < G:
            if taken[h]:
                h += 1
                continue
            if a_s[h] <= remA and b_s[h] <= remB:
                gid_s[h] = k
                col_s[h] = cols
                remA -= a_s[h]
                remB -= b_s[h]
                cols += 1
                h += 1
            else:
                break
        # tail fill: bounded backward search over the smallest nodes for
        # anything that still fits the (remA, remB) leftovers
        j, scan = t, 0
        while j >= h and cols < G and scan < 384:
            if not taken[j] and a_s[j] <= remA and b_s[j] <= remB:
                gid_s[j] = k
                col_s[j] = cols
                remA -= a_s[j]
                remB -= b_s[j]
                cols += 1
                taken[j] = True
                if j == t:
                    t -= 1
            else:
                scan += 1
            j -= 1
        assert cols > 0
        modes.append(1 if y_phase else 0)
        k += 1
    gid = np.empty(n, np.int64)
    col = np.empty(n, np.int64)
    gid[order] = gid_s
    col[order] = col_s
    return gid, col, np.asarray(modes)


def kernel(x, w, a, edge_index):
    global _last_results
    x = np.asarray(x, dtype=np.float32)
    w = np.asarray(w, dtype=np.float32)
    a = np.asarray(a, dtype=np.float32)
    edge_index = np.asarray(edge_index)

    src = edge_index[0].astype(np.int64)
    dst = edge_index[1].astype(np.int64)

    # host: tiny projections + edge scores + ee (bf16, shared with rowsum)
    c_src = (w[:, 0, :] * a[:, :F, 0]).astype(np.float32)   # [H,F]
    c_dst = (w[:, 0, :] * a[:, F:, 0]).astype(np.float32)
    s_src = x @ c_src.T                                     # [N,H]
    s_dst = x @ c_dst.T
    score = s_src[src] + s_dst[dst]                         # [E,H]
    lk = np.where(score > 0, score, 0.2 * score)
    ee = np.exp(-lk, dtype=np.float32)                      # [E,H]
    ee_bf = ee.astype(ml_dtypes.bfloat16)
    ee64 = ee_bf.astype(np.float64)

    rs = np.zeros((H, N_PAD), np.float64)
    for h in range(H):
        rs[h] = np.bincount(src, weights=ee64[:, h], minlength=N_PAD)
    rs[rs == 0] = 1.0

    x_pad = np.zeros((N_PAD, F), np.float32)
    x_pad[:N_NODES] = x
    x_bf = np.ascontiguousarray(x_pad.astype(ml_dtypes.bfloat16))
    iota_np = np.broadcast_to(np.arange(G, dtype=np.float32), (P, G)
                              ).astype(ml_dtypes.bfloat16)

    isB = dst >= HALF
    degA = np.bincount(src, weights=~isB, minlength=N_PAD).astype(np.int64)
    degB = np.bincount(src, weights=isB, minlength=N_PAD).astype(np.int64)
    # a node whose per-side degree exceeds an X-group's side capacity could
    # never be placed; fail loudly instead of corrupting the packing
    assert degA.max() <= 2 * P and degB.max() <= 2 * P, \
        (degA.max(), degB.max())

    # ---- per-device packing ----
    # pass 1: natural X/Y split per device; pass 2: align every device to the
    # max X-group count so the shared schedule pads as little as possible
    ngx_nat = []
    for d in range(NDEV):
        lo = d * NPD
        _, _, modes = _pack_device(degA[lo:lo + NPD], degB[lo:lo + NPD])
        ngx_nat.append(int((modes == 0).sum()))
    ngx_tgt = max(ngx_nat)
    dev_pack = []
    ngx_max = ngy_max = 0
    for d in range(NDEV):
        lo = d * NPD
        gid, col, modes = _pack_device(degA[lo:lo + NPD], degB[lo:lo + NPD],
                                       target_ngx=ngx_tgt)
        ngx = int((modes == 0).sum())
        ngy = int((modes == 1).sum())
        ngx_max = max(ngx_max, ngx)
        ngy_max = max(ngy_max, ngy)
        dev_pack.append((gid, col, modes, ngx))

    # chunk schedule: full X-chunks of 16 groups (+ partial), then Y of 32
    chunks = []
    r = ngx_max
    while r > 0:
        chunks.append((True, min(16, r)))
        r -= min(16, r)
    r = ngy_max
    while r > 0:
        chunks.append((False, min(32, r)))
        r -= min(32, r)
    chunks = tuple(chunks)
    meta, ntiles, idx_cols, eg_cols, out_cols = _chunk_meta(chunks)
    # per-group (schedule-level) lookup tables
    sch_rows = []   # (is_x, chunk_idx, j_in_chunk)
    for ci, (is_x, ng) in enumerate(chunks):
        for j in range(ng):
            sch_rows.append((is_x, ci, j))
    sch_isx = np.array([r[0] for r in sch_rows])
    sch_ci = np.array([r[1] for r in sch_rows])
    sch_j = np.array([r[2] for r in sch_rows])
    m_tile0 = np.array([m["tile0"] for m in meta])
    m_half = np.array([m["half_t"] for m in meta])
    m_out0 = np.array([m["out0"] for m in meta])
    # X-group k (global order) must map to k-th X-slot of the schedule;
    # schedule lists X groups first, so global group id == schedule row.
    gA_base = np.where(sch_isx,
                       m_tile0[sch_ci] + 2 * sch_j,
                       m_tile0[sch_ci] + sch_j)
    gB_base = gA_base + m_half[sch_ci]
    g_out = m_out0[sch_ci] + (sch_j // 8) * 512 + ((sch_j % 8) // 2) * F
    g_prow = ((sch_j % 8) % 2) * C

    in_maps = []
    for d in range(NDEV):
        lo = d * NPD
        gid, col, modes, ngx = dev_pack[d]
        # device group id -> schedule row: X groups k -> k; Y groups k -> (k -
        # ngx) + ngx_max
        m = (src >= lo) & (src < lo + NPD)
        es = src[m] - lo
        ed = dst[m]
        eb = isB[m]
        eee = ee_bf[m]                       # [Ed, H] bf16
        sgid = np.where(modes == 0, np.arange(len(modes)),
                        np.arange(len(modes)) - ngx + ngx_max)
        egid = sgid[gid[es]]
        ecol = col[es]

        # rank of each edge within its (group, side) bucket
        okey = np.lexsort((eb, egid))
        ed_o, eb_o = ed[okey], eb[okey]
        egid_o, ecol_o = egid[okey], ecol[okey]
        eee_o = eee[okey]
        bucket = egid_o * 2 + eb_o
        bchange = np.flatnonzero(np.diff(bucket)) + 1
        starts = np.concatenate([[0], bchange])
        bid = np.zeros(len(bucket), np.int64)
        bid[bchange] = 1
        bid = np.cumsum(bid)
        rank = np.arange(len(bucket)) - starts[bid]

        base = np.where(eb_o, gB_base[egid_o], gA_base[egid_o])
        tt = base + (rank >> 7)
        p = rank & 127

        idx_flat = np.zeros((ntiles, P), np.int16)  # [tile, slot]
        idx_flat[tt, p] = (ed_o - np.where(eb_o, HALF, 0)).astype(np.int16)

        # -1 for the strictly-trailing pad of each gather call: the Q7 trims
        # trailing negatives, skipping those descriptors entirely.
        # per chunk: one [128, t0*8] block; rows [32q,32q+32) hold call q's
        # wrapped idxs ([A0, B0, A1, B1]); flat (tile, slot) -> [i%16, i//16]
        idx_cols_dev = np.zeros((P, idx_cols), np.int16)
        colpos = 0
        for cm in meta:
            t0s = (cm["half_t"] + 1) // 2
            calls = [(0, 0, t0s), (1, 0, t0s),
                     (0, t0s, cm["half_t"] - t0s),
                     (1, t0s, cm["half_t"] - t0s)]
            for q, (side, lo, nt) in enumerate(calls):
                if not nt:
                    continue
                base = cm["tile0"] + side * cm["half_t"]
                fl = idx_flat[base + lo:base + lo + nt].reshape(-1)
                wq = fl.reshape(-1, 16).T    # [16, nt*8]
                idx_cols_dev[32 * q:32 * q + 32, colpos:colpos + nt * 8] = \
                    np.tile(wq, (2, 1))
            colpos += t0s * 8
        assert colpos == idx_cols

        eg_arr = np.zeros((P, eg_cols), ml_dtypes.bfloat16)
        eg_arr[p[:, None], (tt * 5)[:, None] + np.arange(H)[None, :]] = eee_o
        eg_arr[p, tt * 5 + 4] = ecol_o.astype(np.float32).astype(
            ml_dtypes.bfloat16)

        in_maps.append({
            "xtab": x_bf,
            "idxt": idx_cols_dev,
            "egt": np.ascontiguousarray(eg_arr),
            "iota": iota_np,
        })

    if chunks not in _program_cache:
        _program_cache[chunks] = _build_program(chunks)
    nc = _program_cache[chunks]

    res = run_bass_kernel_spmd(nc, in_maps, core_ids=list(range(NDEV)))
    _last_results = res

    # ---- decode ----
    out = np.empty((H, N_PAD, F), np.float32)
    hh = np.arange(H)
    for d in range(NDEV):
        gid, col, modes, ngx = dev_pack[d]
        r = res.results[d]["outt"].astype(np.float32)   # [P, out_cols]
        sgid = np.where(modes == 0, np.arange(len(modes)),
                        np.arange(len(modes)) - ngx + ngx_max)
        node_s = sgid[gid]                              # schedule row per node
        pn = g_prow[node_s] + col                       # h=0 partition row
        fn = g_out[node_s]                              # col base
        idx_p = pn[None, :, None] + (hh * G)[:, None, None]
        idx_f = fn[None, :, None] + np.arange(F)[None, None, :]
        out[:, d * NPD:(d + 1) * NPD, :] = \
            r[np.broadcast_to(idx_p, (H, NPD, F)),
              np.broadcast_to(idx_f, (H, NPD, F))]
    out *= w[:, 0, :][:, None, :]
    out /= rs[:, :, None].astype(np.float32)
    return np.ascontiguousarray(out[:, :N_NODES, :]).astype(np.float32)
